# revision 23
# baseline (speedup 1.0000x reference)
"""Trainium2 Bass kernel for the class-balanced supervised-contrastive loss.

Math (reference semantics, shift-invariant form with constant shift 10):
  l_ij = (f_i . g_j) / T,  T = 0.1, g = [features; centers; features_ood]
  E_ij = exp(l_ij - 10)
  S_i  = sum_{j != i} E_ij / (w_j - eq_ij)        (w_j = class count, eq = label match)
  P_i  = sum_{j != i} eq_ij (l_ij - 10)
  loss = -mean_i( P_i / K_i - log S_i ),  K_i = batch count of class t_i

Device per core (rows globally sorted by class, 512 rows/core, columns permuted
so every eq-match lands in the first WIN=1024 cols):
  psum = f . g + bias1_col   (bias1 = (ln(1/w) - 10)/10, so exp(10*psum) = E/w)
  A_i  = sum_j exp(10*psum)            ACT exp accum_out (per 2048-col group)
  S2_i = sum_{win} eq * E1             DVE masked reduce (window only)
  e1s  = E1_ii                         DVE diagonal gather of the exp tile
The numerator P_i = sum_matched (l-10) is exact host math: f_i . G_{t_i} with
G_c = class-sum of features + center, so only the exp-side stats need the
device; e1s comes from the same SBUF exp tile DVE already reads, so psum
buffers free the moment the ACT exp ends (keeps the 2x2048 psum double
buffer saturated).

All matmuls run as fp8e4 DoubleRow (2 contraction rows per PE pass):
  - main terms: f8h . g8h over K=512 as 2 DR passes of K_eff=256
  - window adds f8l.g8h + f8h.g8l correction terms (dot err ~2.5e-4) plus an
    exact bf16 (hi,lo) K=2 bias matmul
  - cheap (non-window) cols get their bias as a 3-row fp8 DR matmul
    (bh,bm,bl residual cascade, exponent err ~1e-2 -> A err ~1e-4), which is
    also the bank-opening start=True instruction for psum zeroing
Column space per (m-tile) is processed in [128,2048] psum groups (4 banks,
double buffered), each consumed by one wide ACT exp with accum_out.
"""

import ml_dtypes
import numpy as np

import concourse.bass as bass
import concourse.mybir as mybir
import concourse.tile as tile
from concourse.bass_utils import run_bass_kernel_spmd

NCORES = 8
C, TEMP = 1000, 0.1
B, BO, D = 4096, 4096, 512
N = B + C + BO              # 9192
NPAD = 9216                 # 18 * 512
PAD = NPAD - N
NCH = NPAD // 512           # 18 column chunks
RPC = B // NCORES           # 512 rows per core
MT = RPC // 128             # 4 row tiles per core

F32 = mybir.dt.float32
BF16 = mybir.dt.bfloat16
FP8 = mybir.dt.float8e4
DR = mybir.MatmulPerfMode.DoubleRow
ALU = mybir.AluOpType
AF = mybir.ActivationFunctionType
BFNP = ml_dtypes.bfloat16
FP8NP = ml_dtypes.float8_e4m3

# This walrus build accepts only one sync-wait command per engine instruction.
# Move surplus waits onto standalone EventSemaphore instructions just before
# the affected instruction (same engine, so blocking semantics are identical).
_SPLIT_SKIP = ("InstEventSemaphore",)


def _split_multi_waits(nc):
    n = 0
    for f in nc.m.functions:
        for bb in f.blocks:
            new = []
            for ins in bb.instructions:
                si = ins.sync_info
                if (
                    si is not None
                    and si.on_wait
                    and len(si.on_wait) > 1
                    and type(ins).__name__ not in _SPLIT_SKIP
                ):
                    waits = list(si.on_wait)
                    for w in waits[:-1]:
                        es = mybir.InstEventSemaphore(
                            name=f"wsplit_{n}",
                            engine=ins.engine,
                            sync_info=mybir.SyncInfo(on_wait=[w], on_update=[]),
                        )
                        n += 1
                        new.append(es)
                    ins.sync_info = mybir.SyncInfo(
                        on_wait=[waits[-1]], on_update=list(si.on_update)
                    )
                new.append(ins)
            bb.instructions = new
    return n


def _build_nc(wch=2, woff=None):
    """wch = window chunk count; woff = per-m 256-col correction bands, or None
    for full-width correction terms."""
    cch = NCH - wch                     # cheap chunks
    win = 512 * wch
    # cheap chunks packed into psum groups of <=4 chunks
    groups = []
    ch = wch
    while ch < NCH:
        g = list(range(ch, min(ch + 4, NCH)))
        groups.append(g)
        ch += len(g)
    ngrp = len(groups) + 1              # + window group
    nc = bass.Bass()

    # DR layouts: contraction row r = 256*khat + 128*i + p  (pair i, partition p)
    gT8 = nc.declare_dram_parameter("gT8", [128, NCH * 2048], FP8, isOutput=False)
    gT8l = nc.declare_dram_parameter("gT8l", [128, wch * 2048], FP8, isOutput=False)
    fT8 = nc.declare_dram_parameter("fT8", [128, MT * 512], FP8, isOutput=False)
    fT8l = nc.declare_dram_parameter("fT8l", [128, MT * 512], FP8, isOutput=False)
    warm = nc.declare_dram_parameter("warm", [2, 64], FP8, isOutput=False)
    ones8 = nc.declare_dram_parameter("ones8", [2, 256], FP8, isOutput=False)
    b8 = nc.declare_dram_parameter("b8", [2, cch * 1024], FP8, isOutput=False)
    ones2 = nc.declare_dram_parameter("ones2", [2, 128], BF16, isOutput=False)
    bw = nc.declare_dram_parameter("bw", [2, win], BF16, isOutput=False)
    ta = nc.declare_dram_parameter("ta", [128, win], F32, isOutput=False)
    tvec = nc.declare_dram_parameter("tvec", [128, MT], F32, isOutput=False)
    ident = nc.declare_dram_parameter("ident", [128, 128], F32, isOutput=False)
    out = nc.declare_dram_parameter("out", [128, (len(groups) + 1 + wch + 2) * MT], F32, isOutput=True)

    with tile.TileContext(nc) as tc:
        with (
            tc.tile_pool(name="const", bufs=1) as const,
            tc.tile_pool(name="stats", bufs=1) as stats,
            tc.tile_pool(name="gt", bufs=8) as gtp,
            tc.tile_pool(name="e1c", bufs=3) as e1cp,
            tc.tile_pool(name="e1w", bufs=2) as e1wp,
            tc.tile_pool(name="psum", bufs=2, space="PSUM") as psp,
        ):
            ft = const.tile([128, MT, 2, 2, 128], FP8)
            ftl = const.tile([128, MT, 2, 2, 128], FP8)
            warm_sb = const.tile([2, 2, 32], FP8)
            warm_o = const.tile([2, 32], F32)
            ones8_sb = const.tile([2, 2, 128], FP8)
            b8_sb = const.tile([2, cch, 2, 512], FP8)
            ones2_sb = const.tile([2, 128], BF16)
            bw_sb = const.tile([2, win], BF16)
            ta_sb = const.tile([128, win], F32)
            tvec_sb = const.tile([128, MT], F32)
            ident_sb = const.tile([128, 128], F32)
            gl = const.tile([128, wch, 2, 2, 512], FP8)

            nc.sync.dma_start(out=warm_sb[:], in_=warm[:])
            nc.scalar.dma_start(out=ones8_sb[:], in_=ones8[:])
            nc.gpsimd.dma_start(out=ft[:], in_=fT8[:])
            # Exp table preload off the critical path
            nc.scalar.activation(warm_o[:], warm_sb[:, 0], AF.Exp, scale=1.0)

            # per m: ngrp A-partials | wch S2 parts | e1s diag | m0 split-A extra
            acc = [stats.tile([128, ngrp + wch + 2], F32, name=f"acc{m}") for m in range(MT)]

            group_tiles = {}

            def load_group(gi, g, spread=False):
                beng = nc.sync
                beng.dma_start(
                    out=b8_sb[:, g[0] - wch : g[-1] + 1 - wch],
                    in_=b8[:, 1024 * (g[0] - wch) : 1024 * (g[-1] + 1 - wch)],
                )
                gts = []
                for ci, ch in enumerate(g):
                    gt = gtp.tile([128, 2, 2, 512], FP8, name=f"g{ch}", tag="gt")
                    eng = nc.sync
                    if spread:
                        eng = (nc.gpsimd, nc.gpsimd, nc.sync, nc.gpsimd)[ci % 4]
                    eng.dma_start(
                        out=gt[:], in_=gT8[:, 2048 * ch : 2048 * (ch + 1)]
                    )
                    gts.append(gt)
                group_tiles[gi] = gts

            def cheap_unit(gi, g, m, warmups=0, split=False):
                gts = group_tiles[gi]
                ps = psp.tile([128, 2048], F32)
                # PE warmup/p-state ramp spins while the first DMAs land
                for _ in range(warmups):
                    nc.tensor.matmul(
                        ps[:, 0:128], ones8_sb[:], ones8_sb[:],
                        start=True, stop=True, perf_mode=DR, skip_group_check=True,
                    )
                for ci, ch in enumerate(g):
                    cs = slice(512 * ci, 512 * (ci + 1))
                    nc.tensor.matmul(
                        ps[:, cs], ones8_sb[:], b8_sb[:, ch - wch],
                        start=True, stop=False, perf_mode=DR,
                    )
                    for k in range(2):
                        nc.tensor.matmul(
                            ps[:, cs], ft[:, m, k], gts[ci][:, k],
                            start=False, stop=(k == 1), perf_mode=DR,
                        )
                    if split and ci == 1:
                        # early half-exp so ACT starts before chunks 3-4 land
                        e1a = e1cp.tile([128, 1024], BF16, tag="e1a")
                        nc.scalar.activation(
                            e1a[:], ps[:, :1024], AF.Exp, scale=10.0,
                            accum_out=acc[m][:, ngrp + wch + 1 : ngrp + wch + 2],
                        )
                w = 512 * len(g)
                off = 1024 if split else 0
                e1c = e1cp.tile([128, w - off], BF16, tag="e1c")
                nc.scalar.activation(
                    e1c[:], ps[:, off:w], AF.Exp, scale=10.0,
                    accum_out=acc[m][:, 1 + gi : 2 + gi],
                )

            load_group(0, groups[0], spread=True)
            cheap_unit(0, groups[0], 0, warmups=2, split=True)
            for m in range(1, MT):
                nc.vector.memset(acc[m][:, ngrp + wch + 1 : ngrp + wch + 2], 0.0)
                cheap_unit(0, groups[0], m)

            # window DMAs land while group 0/1 compute
            nc.scalar.dma_start(out=ftl[:], in_=fT8l[:])
            nc.scalar.dma_start(out=ones2_sb[:], in_=ones2[:])
            nc.scalar.dma_start(out=bw_sb[:], in_=bw[:])
            nc.gpsimd.dma_start(out=ta_sb[:], in_=ta[:])
            nc.gpsimd.dma_start(out=tvec_sb[:], in_=tvec[:])
            nc.gpsimd.dma_start(out=ident_sb[:], in_=ident[:])
            gw = []
            for ch in range(wch):
                nc.gpsimd.dma_start(
                    out=gl[:, ch], in_=gT8l[:, 2048 * ch : 2048 * (ch + 1)]
                )
                gt = gtp.tile([128, 2, 2, 512], FP8, name=f"gw{ch}", tag="gtw")
                nc.gpsimd.dma_start(out=gt[:], in_=gT8[:, 2048 * ch : 2048 * (ch + 1)])
                gw.append(gt)

            load_group(1, groups[1])
            for m in range(MT):
                # interleave: cheap unit first so its psum fill leads the pair
                cheap_unit(1, groups[1], m)
                ps = psp.tile([128, 2048], F32)
                for ch in range(wch):
                    cs = slice(512 * ch, 512 * (ch + 1))
                    nc.tensor.matmul(
                        ps[:, cs], ones2_sb[:], bw_sb[:, cs],
                        start=True, stop=False,
                    )
                    if woff is None:
                        co, cn = 0, 512
                    elif ch == 0:
                        co, cn = woff[m], 256
                    else:
                        co, cn = 0, 256
                    ccs = slice(512 * ch + co, 512 * ch + co + cn)
                    for lhs, rhss in (
                        (ftl, (gw[ch][:, 0], gw[ch][:, 1])),
                        (ft, (gl[:, ch, 0], gl[:, ch, 1])),
                    ):
                        for k in range(2):
                            nc.tensor.matmul(
                                ps[:, ccs], lhs[:, m, k],
                                rhss[k][:, :, co : co + cn],
                                start=False, stop=False, perf_mode=DR,
                            )
                    for k in range(2):
                        nc.tensor.matmul(
                            ps[:, cs], ft[:, m, k], gw[ch][:, k],
                            start=False, stop=(k == 1), perf_mode=DR,
                        )
                e1w = e1wp.tile([128, win], F32, tag="e1w")
                nc.scalar.activation(e1w[:], ps[:, :win], AF.Exp, scale=10.0)
                nc.vector.tensor_reduce(
                    acc[m][:, 0:1], e1w[:], mybir.AxisListType.X, ALU.add,
                )
                # self term e1s: gather the diagonal of the SBUF exp tile (no
                # psum read, so the psum buffer frees as soon as the exp ends)
                sd = e1wp.tile([128, 128], F32, tag="scrd")
                nc.vector.scalar_tensor_tensor(
                    out=sd[:], in0=ident_sb[:], scalar=1.0,
                    in1=e1w[:, 128 * m : 128 * (m + 1)],
                    op0=ALU.mult, op1=ALU.mult,
                    accum_out=acc[m][:, ngrp + wch : ngrp + wch + 1],
                )
                for ch in range(wch):
                    cs = slice(512 * ch, 512 * (ch + 1))
                    sc = e1wp.tile([128, 512], F32, tag="scr2")
                    nc.vector.scalar_tensor_tensor(
                        out=sc[:], in0=ta_sb[:, cs], scalar=tvec_sb[:, m : m + 1],
                        in1=e1w[:, cs], op0=ALU.is_equal, op1=ALU.mult,
                        accum_out=acc[m][:, ngrp + ch : ngrp + ch + 1],
                    )


            # -- remaining cheap groups -------------------------------------
            for gi, g in enumerate(groups[2:], start=2):
                load_group(gi, g)
                for m in range(MT):
                    cheap_unit(gi, g, m)

            na = ngrp + wch + 2
            for m in range(MT):
                nc.sync.dma_start(out=out[:, na * m : na * (m + 1)], in_=acc[m][:])
    _split_multi_waits(nc)
    return nc


_nc_by_cfg = {}


def _get_nc(wch, woff):
    key = (wch, woff)
    if key not in _nc_by_cfg:
        _nc_by_cfg[key] = _build_nc(wch, woff)
    return _nc_by_cfg[key]


def _fp8_cascade(x, n):
    """Split x into n fp8 rows summing (in f32) to ~x."""
    rows = []
    rem = np.asarray(x, np.float64).copy()
    for _ in range(n):
        h = rem.astype(FP8NP)
        rows.append(h)
        rem = rem - h.astype(np.float64)
    return rows


def _dr_tile(x):
    """[ncols, 512] fp8 -> [128, ncols/512 * 2048] in the DR chunk layout:
    [p, ch*2048 + (khat*2 + i)*512 + j] = x[512*ch + j, 256*khat + 128*i + p]."""
    nch = x.shape[0] // 512
    xt = np.ascontiguousarray(x.T)                  # [512, ncols]
    return np.ascontiguousarray(
        xt.reshape(2, 2, 128, nch, 512).transpose(2, 3, 0, 1, 4).reshape(128, -1)
    )


def _dr_tile_f(x):
    """[512 rows, 512 dims] fp8 -> [128, MT*512] stationary layout:
    [p, ((m*2 + khat)*2 + i)*128 + q] = x[128*m + q, 256*khat + 128*i + p]."""
    xt = np.ascontiguousarray(x.T)                  # [512 dims, 512 rows]
    return np.ascontiguousarray(
        xt.reshape(2, 2, 128, MT, 128).transpose(2, 3, 0, 1, 4).reshape(128, -1)
    )


def _prepare(centers1, features, targets, features_ood, pseudo_target_ood):
    """Host-side prep: sort rows by class, shard contiguously, and per core
    permute the g columns to [own 512 | matched | rest | ood | pad] so all
    eq-matches (and the diagonal, at window column 128m+p) land in the first
    WIN columns."""
    centers1 = np.asarray(centers1, np.float32)
    features = np.asarray(features, np.float32)
    features_ood = np.asarray(features_ood, np.float32)
    targets = np.asarray(targets).astype(np.int64)
    pseudo = np.asarray(pseudo_target_ood).astype(np.int64)

    tac = np.concatenate([targets, np.arange(C), pseudo])
    w_full = np.bincount(tac, minlength=C).astype(np.float64)

    # class-id label per g row (incl. centers/ood), and bias per g row
    lab = np.concatenate([targets, np.arange(C), np.full(BO, C, np.int64),
                          np.full(PAD, -1, np.int64)])
    bias1 = np.full(NPAD, -20.0, np.float64)
    bias1[:N] = -(np.log(w_full[tac]) + 10.0) / 10.0

    g = np.concatenate(
        [features, centers1, features_ood, np.zeros((PAD, D), np.float32)], axis=0
    )
    g8h = g.astype(FP8NP)
    g8l = (g - g8h.astype(np.float32)).astype(FP8NP)

    row_perm = np.argsort(targets, kind="stable")
    t_sorted = targets[row_perm]

    # per-core column permutations
    perms = []
    len_matched = []
    win_need = 1
    all_batch = np.arange(B)
    for c in range(NCORES):
        own = row_perm[RPC * c : RPC * (c + 1)]            # sorted by class
        tset = np.zeros(C + 1, bool)
        tset[t_sorted[RPC * c : RPC * (c + 1)]] = True
        in_own = np.zeros(B, bool)
        in_own[own] = True
        match_b = all_batch[tset[targets] & ~in_own]       # other cores' rows, own classes
        match_c = B + np.flatnonzero(tset[:C])             # centers of own classes
        matched = np.concatenate([match_b, match_c])
        rest_mask = np.ones(B + C, bool)
        rest_mask[own] = False
        rest_mask[matched] = False
        rest = np.flatnonzero(rest_mask)
        perm = np.concatenate(
            [own, matched, rest,
             np.arange(B + C, N),                          # ood
             np.arange(N, NPAD)]                           # pad
        )
        assert perm.shape == (NPAD,)
        perms.append(perm)
        len_matched.append(len(matched))
        win_need = max(win_need, RPC + len(matched))

    wch = max(2, -(-win_need // 512))
    win = 512 * wch
    cch = NCH - wch

    # window bias rows (bf16 hi+lo)
    bh_all = bias1.astype(BFNP)
    bl_all = (bias1 - bh_all.astype(np.float64)).astype(BFNP)
    b8_rows = _fp8_cascade(bias1, 3)                       # bh, bm, bl fp8

    ones8_host = np.zeros((2, 2, 128), np.float32)
    ones8_host[0, 0] = 1.0
    ones8_host[0, 1] = 1.0
    ones8_host[1, 0] = 1.0
    ones2_host = np.ones((2, 128), np.float32)
    ident = np.eye(128, dtype=np.float32)

    in_maps = []
    for c in range(NCORES):
        perm = perms[c]
        own = perm[:RPC]
        f8h = g8h[own]                                     # [512, 512] fp8
        f8l = g8l[own]
        # cheap bias rows: [p, cch_idx, i, j] with (0,0)=bh (0,1)=bm (1,0)=bl
        b8c = np.zeros((2, cch, 2, 512), FP8NP)
        pc = perm[win:].reshape(cch, 512)
        b8c[0, :, 0] = b8_rows[0][pc]
        b8c[0, :, 1] = b8_rows[1][pc]
        b8c[1, :, 0] = b8_rows[2][pc]
        bw_c = np.stack([bh_all[perm[:win]], bl_all[perm[:win]]])
        ta_p = lab[perm[:win]].astype(np.float32)
        in_maps.append(
            {
                "warm": np.full((2, 64), 0.125, FP8NP),
                "gT8": _dr_tile(g8h[perm]),
                "gT8l": _dr_tile(g8l[perm[:win]]),
                "fT8": _dr_tile_f(f8h),
                "fT8l": _dr_tile_f(f8l),
                "ones8": np.ascontiguousarray(ones8_host.reshape(2, 256).astype(FP8NP)),
                "b8": np.ascontiguousarray(b8c.reshape(2, cch * 1024)),
                "ones2": np.ascontiguousarray(ones2_host.astype(BFNP)),
                "bw": np.ascontiguousarray(bw_c.astype(BFNP)),
                "ta": np.ascontiguousarray(np.broadcast_to(ta_p, (128, win))),
                "tvec": np.ascontiguousarray(
                    t_sorted[RPC * c : RPC * (c + 1)].reshape(MT, 128).T.astype(np.float32)
                ),
                "ident": ident,
            }
        )

    # host-side numerator: P_i = 10*(f_i . G_{t_i} - f_i . f_i) - 10*K_i
    # with G_c = sum of batch features of class c + center_c (exact, f64)
    f64 = features.astype(np.float64)
    G = centers1.astype(np.float64).copy()
    np.add.at(G, targets, f64)
    dots = np.einsum("ij,ij->i", f64, G[targets])
    self_dot = np.einsum("ij,ij->i", f64, f64)
    P_half = dots - self_dot                       # sum over matched != self of r

    # narrow correction bands: ch0 matches stay in [woff_m, woff_m+256) of the
    # own-rows chunk, ch1 matches in its first 256 cols
    WOFF = (0, 64, 192, 256)
    woff = WOFF
    if wch != 2:
        woff = None
    else:
        for c in range(NCORES):
            tc_ = t_sorted[RPC * c : RPC * (c + 1)]
            if len(perms[c]) and (RPC + len_matched[c]) > 512 + 256:
                woff = None
                break
            for m in range(MT):
                cmin, cmax = tc_[128 * m], tc_[128 * m + 127]
                lo = np.searchsorted(tc_, cmin, side="left")
                hi = np.searchsorted(tc_, cmax, side="right")
                if not (WOFF[m] <= lo and hi <= WOFF[m] + 256):
                    woff = None
                    break
            if woff is None:
                break

    ncheap_groups = -(-(NCH - wch) // 4)
    host = {"t_sorted": t_sorted, "w_full": w_full, "wch": wch, "woff": woff,
            "ngrp": ncheap_groups + 1, "P_half": P_half[row_perm]}
    return in_maps, host


def _combine(results, host):
    t_sorted = host["t_sorted"]
    w_full = host["w_full"]
    cnt_batch = np.bincount(t_sorted, minlength=C).astype(np.float64)

    ngrp = host["ngrp"]
    wch = host["wch"]
    na = ngrp + wch + 2
    A = np.empty(B)
    S2 = np.empty(B)
    e1s = np.empty(B)
    for c in range(NCORES):
        o = np.asarray(results[c]["out"], np.float64)
        for m in range(MT):
            rs = slice(RPC * c + 128 * m, RPC * c + 128 * (m + 1))
            a = o[:, na * m : na * (m + 1)]
            A[rs] = a[:, 0:ngrp].sum(axis=1) + a[:, ngrp + wch + 1]
            S2[rs] = a[:, ngrp : ngrp + wch].sum(axis=1)
            e1s[rs] = a[:, ngrp + wch]

    ws = w_full[t_sorted]
    K = cnt_batch[t_sorted]
    ds_ = 1.0 / (ws - 1.0) - 1.0 / ws
    S = A - e1s + ds_ * ws * (S2 - e1s)
    P = 10.0 * host["P_half"] - 10.0 * K
    val = P / K - np.log(S)
    return np.float32(-val.mean())


def _run(inputs, trace=False, **kw):
    in_maps, host = _prepare(**inputs)
    nc = _get_nc(host["wch"], host["woff"])
    res = run_bass_kernel_spmd(nc, in_maps, list(range(NCORES)), trace=trace, **kw)
    loss = _combine(res.results, host)
    return loss, res


def kernel(**inputs):
    loss, _ = _run(inputs)
    return loss


# revision 24
# speedup vs baseline: 1.0099x; 1.0099x over previous
"""Trainium2 Bass kernel for the class-balanced supervised-contrastive loss.

Math (reference semantics, shift-invariant form with constant shift 10):
  l_ij = (f_i . g_j) / T,  T = 0.1, g = [features; centers; features_ood]
  E_ij = exp(l_ij - 10)
  S_i  = sum_{j != i} E_ij / (w_j - eq_ij)        (w_j = class count, eq = label match)
  P_i  = sum_{j != i} eq_ij (l_ij - 10)
  loss = -mean_i( P_i / K_i - log S_i ),  K_i = batch count of class t_i

Device per core (rows globally sorted by class, 512 rows/core, columns permuted
so every eq-match lands in the first WIN=1024 cols):
  psum = f . g + bias1_col   (bias1 = (ln(1/w) - 10)/10, so exp(10*psum) = E/w)
  A_i  = sum_j exp(10*psum)            ACT exp accum_out (per 2048-col group)
  S2_i = sum_{win} eq * E1             DVE masked reduce (window only)
  e1s  = E1_ii                         DVE diagonal gather of the exp tile
The numerator P_i = sum_matched (l-10) is exact host math: f_i . G_{t_i} with
G_c = class-sum of features + center, so only the exp-side stats need the
device; e1s comes from the same SBUF exp tile DVE already reads, so psum
buffers free the moment the ACT exp ends (keeps the 2x2048 psum double
buffer saturated).

All matmuls run as fp8e4 DoubleRow (2 contraction rows per PE pass):
  - main terms: f8h . g8h over K=512 as 2 DR passes of K_eff=256
  - window adds f8l.g8h + f8h.g8l correction terms (dot err ~2.5e-4) plus an
    exact bf16 (hi,lo) K=2 bias matmul
  - cheap (non-window) cols get their bias as a 3-row fp8 DR matmul
    (bh,bm,bl residual cascade, exponent err ~1e-2 -> A err ~1e-4), which is
    also the bank-opening start=True instruction for psum zeroing
Column space per (m-tile) is processed in [128,2048] psum groups (4 banks,
double buffered), each consumed by one wide ACT exp with accum_out.
"""

import ml_dtypes
import numpy as np

import concourse.bass as bass
import concourse.mybir as mybir
import concourse.tile as tile
from concourse.bass_utils import run_bass_kernel_spmd

NCORES = 8
C, TEMP = 1000, 0.1
B, BO, D = 4096, 4096, 512
N = B + C + BO              # 9192
NPAD = 9216                 # 18 * 512
PAD = NPAD - N
NCH = NPAD // 512           # 18 column chunks
RPC = B // NCORES           # 512 rows per core
MT = RPC // 128             # 4 row tiles per core

F32 = mybir.dt.float32
BF16 = mybir.dt.bfloat16
FP8 = mybir.dt.float8e4
DR = mybir.MatmulPerfMode.DoubleRow
ALU = mybir.AluOpType
AF = mybir.ActivationFunctionType
BFNP = ml_dtypes.bfloat16
FP8NP = ml_dtypes.float8_e4m3

# This walrus build accepts only one sync-wait command per engine instruction.
# Move surplus waits onto standalone EventSemaphore instructions just before
# the affected instruction (same engine, so blocking semantics are identical).
_SPLIT_SKIP = ("InstEventSemaphore",)


def _split_multi_waits(nc):
    n = 0
    for f in nc.m.functions:
        for bb in f.blocks:
            new = []
            for ins in bb.instructions:
                si = ins.sync_info
                if (
                    si is not None
                    and si.on_wait
                    and len(si.on_wait) > 1
                    and type(ins).__name__ not in _SPLIT_SKIP
                ):
                    waits = list(si.on_wait)
                    for w in waits[:-1]:
                        es = mybir.InstEventSemaphore(
                            name=f"wsplit_{n}",
                            engine=ins.engine,
                            sync_info=mybir.SyncInfo(on_wait=[w], on_update=[]),
                        )
                        n += 1
                        new.append(es)
                    ins.sync_info = mybir.SyncInfo(
                        on_wait=[waits[-1]], on_update=list(si.on_update)
                    )
                new.append(ins)
            bb.instructions = new
    return n


def _build_nc(wch=2, woff=None):
    """wch = window chunk count; woff = per-m 256-col correction bands, or None
    for full-width correction terms."""
    cch = NCH - wch                     # cheap chunks
    win = 512 * wch
    # cheap chunks packed into psum groups of <=4 chunks
    groups = []
    ch = wch
    while ch < NCH:
        g = list(range(ch, min(ch + 4, NCH)))
        groups.append(g)
        ch += len(g)
    ngrp = len(groups) + 1              # + window group
    nc = bass.Bass()

    # DR layouts: contraction row r = 256*khat + 128*i + p  (pair i, partition p)
    gT8 = nc.declare_dram_parameter("gT8", [128, NCH * 2048], FP8, isOutput=False)
    gT8l = nc.declare_dram_parameter("gT8l", [128, wch * 2048], FP8, isOutput=False)
    fT8 = nc.declare_dram_parameter("fT8", [128, MT * 512], FP8, isOutput=False)
    fT8l = nc.declare_dram_parameter("fT8l", [128, MT * 512], FP8, isOutput=False)
    warm = nc.declare_dram_parameter("warm", [2, 64], FP8, isOutput=False)
    ones8 = nc.declare_dram_parameter("ones8", [2, 256], FP8, isOutput=False)
    b8 = nc.declare_dram_parameter("b8", [2, cch * 1024], FP8, isOutput=False)
    ones2 = nc.declare_dram_parameter("ones2", [2, 128], BF16, isOutput=False)
    bw = nc.declare_dram_parameter("bw", [2, win], BF16, isOutput=False)
    ta = nc.declare_dram_parameter("ta", [128, win], F32, isOutput=False)
    tvec = nc.declare_dram_parameter("tvec", [128, MT], F32, isOutput=False)
    ident = nc.declare_dram_parameter("ident", [128, 128], F32, isOutput=False)
    out = nc.declare_dram_parameter("out", [128, (len(groups) + 1 + wch + 1) * MT], F32, isOutput=True)

    with tile.TileContext(nc) as tc:
        with (
            tc.tile_pool(name="const", bufs=1) as const,
            tc.tile_pool(name="stats", bufs=1) as stats,
            tc.tile_pool(name="gt", bufs=8) as gtp,
            tc.tile_pool(name="e1c", bufs=3) as e1cp,
            tc.tile_pool(name="e1w", bufs=2) as e1wp,
            tc.tile_pool(name="psum", bufs=2, space="PSUM") as psp,
        ):
            ft = const.tile([128, MT, 2, 2, 128], FP8)
            ftl = const.tile([128, MT, 2, 2, 128], FP8)
            warm_sb = const.tile([2, 2, 32], FP8)
            warm_o = const.tile([2, 32], F32)
            ones8_sb = const.tile([2, 2, 128], FP8)
            b8_sb = const.tile([2, cch, 2, 512], FP8)
            ones2_sb = const.tile([2, 128], BF16)
            bw_sb = const.tile([2, win], BF16)
            ta_sb = const.tile([128, win], F32)
            tvec_sb = const.tile([128, MT], F32)
            ident_sb = const.tile([128, 128], F32)
            gl = const.tile([128, wch, 2, 2, 512], FP8)

            nc.sync.dma_start(out=warm_sb[:], in_=warm[:])
            nc.scalar.dma_start(out=ones8_sb[:], in_=ones8[:])
            nc.gpsimd.dma_start(out=ft[:], in_=fT8[:])
            # Exp table preload off the critical path
            nc.scalar.activation(warm_o[:], warm_sb[:, 0], AF.Exp, scale=1.0)

            # per m: ngrp A-partials | wch S2 parts | e1s diag
            acc = [stats.tile([128, ngrp + wch + 1], F32, name=f"acc{m}") for m in range(MT)]

            group_tiles = {}

            def load_group(gi, g, spread=False):
                beng = nc.sync
                beng.dma_start(
                    out=b8_sb[:, g[0] - wch : g[-1] + 1 - wch],
                    in_=b8[:, 1024 * (g[0] - wch) : 1024 * (g[-1] + 1 - wch)],
                )
                gts = []
                for ci, ch in enumerate(g):
                    gt = gtp.tile([128, 2, 2, 512], FP8, name=f"g{ch}", tag="gt")
                    eng = nc.sync
                    if spread:
                        eng = (nc.gpsimd, nc.gpsimd, nc.sync, nc.gpsimd)[ci % 4]
                    eng.dma_start(
                        out=gt[:], in_=gT8[:, 2048 * ch : 2048 * (ch + 1)]
                    )
                    gts.append(gt)
                group_tiles[gi] = gts

            def cheap_unit(gi, g, m, warmups=0):
                gts = group_tiles[gi]
                ps = psp.tile([128, 2048], F32)
                # PE warmup/p-state ramp spins while the first DMAs land
                for _ in range(warmups):
                    nc.tensor.matmul(
                        ps[:, 0:128], ones8_sb[:], ones8_sb[:],
                        start=True, stop=True, perf_mode=DR, skip_group_check=True,
                    )
                for ci, ch in enumerate(g):
                    cs = slice(512 * ci, 512 * (ci + 1))
                    nc.tensor.matmul(
                        ps[:, cs], ones8_sb[:], b8_sb[:, ch - wch],
                        start=True, stop=False, perf_mode=DR,
                    )
                    for k in range(2):
                        nc.tensor.matmul(
                            ps[:, cs], ft[:, m, k], gts[ci][:, k],
                            start=False, stop=(k == 1), perf_mode=DR,
                        )
                e1c = e1cp.tile([128, 512 * len(g)], BF16, tag="e1c")
                nc.scalar.activation(
                    e1c[:], ps[:, : 512 * len(g)], AF.Exp, scale=10.0,
                    accum_out=acc[m][:, 1 + gi : 2 + gi],
                )

            load_group(0, groups[0], spread=True)
            cheap_unit(0, groups[0], 0, warmups=2)
            for m in range(1, MT):
                cheap_unit(0, groups[0], m)

            # window DMAs land while group 0/1 compute
            nc.scalar.dma_start(out=ftl[:], in_=fT8l[:])
            nc.scalar.dma_start(out=ones2_sb[:], in_=ones2[:])
            nc.scalar.dma_start(out=bw_sb[:], in_=bw[:])
            nc.gpsimd.dma_start(out=ta_sb[:], in_=ta[:])
            nc.gpsimd.dma_start(out=tvec_sb[:], in_=tvec[:])
            nc.gpsimd.dma_start(out=ident_sb[:], in_=ident[:])
            gw = []
            for ch in range(wch):
                nc.gpsimd.dma_start(
                    out=gl[:, ch], in_=gT8l[:, 2048 * ch : 2048 * (ch + 1)]
                )
                gt = gtp.tile([128, 2, 2, 512], FP8, name=f"gw{ch}", tag="gtw")
                nc.gpsimd.dma_start(out=gt[:], in_=gT8[:, 2048 * ch : 2048 * (ch + 1)])
                gw.append(gt)

            load_group(1, groups[1])
            for m in range(MT):
                # interleave: cheap unit first so its psum fill leads the pair
                cheap_unit(1, groups[1], m)
                ps = psp.tile([128, 2048], F32)
                for ch in range(wch):
                    cs = slice(512 * ch, 512 * (ch + 1))
                    nc.tensor.matmul(
                        ps[:, cs], ones2_sb[:], bw_sb[:, cs],
                        start=True, stop=False,
                    )
                    if woff is None:
                        co, cn = 0, 512
                    elif ch == 0:
                        co, cn = woff[m], 256
                    else:
                        co, cn = 0, 256
                    ccs = slice(512 * ch + co, 512 * ch + co + cn)
                    for lhs, rhss in (
                        (ftl, (gw[ch][:, 0], gw[ch][:, 1])),
                        (ft, (gl[:, ch, 0], gl[:, ch, 1])),
                    ):
                        for k in range(2):
                            nc.tensor.matmul(
                                ps[:, ccs], lhs[:, m, k],
                                rhss[k][:, :, co : co + cn],
                                start=False, stop=False, perf_mode=DR,
                            )
                    for k in range(2):
                        nc.tensor.matmul(
                            ps[:, cs], ft[:, m, k], gw[ch][:, k],
                            start=False, stop=(k == 1), perf_mode=DR,
                        )
                e1w = e1wp.tile([128, win], F32, tag="e1w")
                nc.scalar.activation(e1w[:], ps[:, :win], AF.Exp, scale=10.0)
                nc.vector.tensor_reduce(
                    acc[m][:, 0:1], e1w[:], mybir.AxisListType.X, ALU.add,
                )
                # self term e1s: gather the diagonal of the SBUF exp tile (no
                # psum read, so the psum buffer frees as soon as the exp ends)
                sd = e1wp.tile([128, 128], F32, tag="scrd")
                nc.vector.scalar_tensor_tensor(
                    out=sd[:], in0=ident_sb[:], scalar=1.0,
                    in1=e1w[:, 128 * m : 128 * (m + 1)],
                    op0=ALU.mult, op1=ALU.mult,
                    accum_out=acc[m][:, ngrp + wch : ngrp + wch + 1],
                )
                for ch in range(wch):
                    cs = slice(512 * ch, 512 * (ch + 1))
                    sc = e1wp.tile([128, 512], F32, tag="scr2")
                    nc.vector.scalar_tensor_tensor(
                        out=sc[:], in0=ta_sb[:, cs], scalar=tvec_sb[:, m : m + 1],
                        in1=e1w[:, cs], op0=ALU.is_equal, op1=ALU.mult,
                        accum_out=acc[m][:, ngrp + ch : ngrp + ch + 1],
                    )


            # -- remaining cheap groups -------------------------------------
            for gi, g in enumerate(groups[2:], start=2):
                load_group(gi, g)
                for m in range(MT):
                    cheap_unit(gi, g, m)

            na = ngrp + wch + 1
            for m in range(MT):
                nc.sync.dma_start(out=out[:, na * m : na * (m + 1)], in_=acc[m][:])
    _split_multi_waits(nc)
    return nc


_nc_by_cfg = {}


def _get_nc(wch, woff):
    key = (wch, woff)
    if key not in _nc_by_cfg:
        _nc_by_cfg[key] = _build_nc(wch, woff)
    return _nc_by_cfg[key]


def _fp8_cascade(x, n):
    """Split x into n fp8 rows summing (in f32) to ~x."""
    rows = []
    rem = np.asarray(x, np.float64).copy()
    for _ in range(n):
        h = rem.astype(FP8NP)
        rows.append(h)
        rem = rem - h.astype(np.float64)
    return rows


def _dr_tile(x):
    """[ncols, 512] fp8 -> [128, ncols/512 * 2048] in the DR chunk layout:
    [p, ch*2048 + (khat*2 + i)*512 + j] = x[512*ch + j, 256*khat + 128*i + p]."""
    nch = x.shape[0] // 512
    xt = np.ascontiguousarray(x.T)                  # [512, ncols]
    return np.ascontiguousarray(
        xt.reshape(2, 2, 128, nch, 512).transpose(2, 3, 0, 1, 4).reshape(128, -1)
    )


def _dr_tile_f(x):
    """[512 rows, 512 dims] fp8 -> [128, MT*512] stationary layout:
    [p, ((m*2 + khat)*2 + i)*128 + q] = x[128*m + q, 256*khat + 128*i + p]."""
    xt = np.ascontiguousarray(x.T)                  # [512 dims, 512 rows]
    return np.ascontiguousarray(
        xt.reshape(2, 2, 128, MT, 128).transpose(2, 3, 0, 1, 4).reshape(128, -1)
    )


def _prepare(centers1, features, targets, features_ood, pseudo_target_ood):
    """Host-side prep: sort rows by class, shard contiguously, and per core
    permute the g columns to [own 512 | matched | rest | ood | pad] so all
    eq-matches (and the diagonal, at window column 128m+p) land in the first
    WIN columns."""
    centers1 = np.asarray(centers1, np.float32)
    features = np.asarray(features, np.float32)
    features_ood = np.asarray(features_ood, np.float32)
    targets = np.asarray(targets).astype(np.int64)
    pseudo = np.asarray(pseudo_target_ood).astype(np.int64)

    tac = np.concatenate([targets, np.arange(C), pseudo])
    w_full = np.bincount(tac, minlength=C).astype(np.float64)

    # class-id label per g row (incl. centers/ood), and bias per g row
    lab = np.concatenate([targets, np.arange(C), np.full(BO, C, np.int64),
                          np.full(PAD, -1, np.int64)])
    bias1 = np.full(NPAD, -20.0, np.float64)
    bias1[:N] = -(np.log(w_full[tac]) + 10.0) / 10.0

    g = np.concatenate(
        [features, centers1, features_ood, np.zeros((PAD, D), np.float32)], axis=0
    )
    g8h = g.astype(FP8NP)
    g8l = (g - g8h.astype(np.float32)).astype(FP8NP)

    row_perm = np.argsort(targets, kind="stable")
    t_sorted = targets[row_perm]

    # per-core column permutations
    perms = []
    len_matched = []
    win_need = 1
    all_batch = np.arange(B)
    for c in range(NCORES):
        own = row_perm[RPC * c : RPC * (c + 1)]            # sorted by class
        tset = np.zeros(C + 1, bool)
        tset[t_sorted[RPC * c : RPC * (c + 1)]] = True
        in_own = np.zeros(B, bool)
        in_own[own] = True
        match_b = all_batch[tset[targets] & ~in_own]       # other cores' rows, own classes
        match_c = B + np.flatnonzero(tset[:C])             # centers of own classes
        matched = np.concatenate([match_b, match_c])
        rest_mask = np.ones(B + C, bool)
        rest_mask[own] = False
        rest_mask[matched] = False
        rest = np.flatnonzero(rest_mask)
        perm = np.concatenate(
            [own, matched, rest,
             np.arange(B + C, N),                          # ood
             np.arange(N, NPAD)]                           # pad
        )
        assert perm.shape == (NPAD,)
        perms.append(perm)
        len_matched.append(len(matched))
        win_need = max(win_need, RPC + len(matched))

    wch = max(2, -(-win_need // 512))
    win = 512 * wch
    cch = NCH - wch

    # window bias rows (bf16 hi+lo)
    bh_all = bias1.astype(BFNP)
    bl_all = (bias1 - bh_all.astype(np.float64)).astype(BFNP)
    b8_rows = _fp8_cascade(bias1, 3)                       # bh, bm, bl fp8

    ones8_host = np.zeros((2, 2, 128), np.float32)
    ones8_host[0, 0] = 1.0
    ones8_host[0, 1] = 1.0
    ones8_host[1, 0] = 1.0
    ones2_host = np.ones((2, 128), np.float32)
    ident = np.eye(128, dtype=np.float32)

    in_maps = []
    for c in range(NCORES):
        perm = perms[c]
        own = perm[:RPC]
        f8h = g8h[own]                                     # [512, 512] fp8
        f8l = g8l[own]
        # cheap bias rows: [p, cch_idx, i, j] with (0,0)=bh (0,1)=bm (1,0)=bl
        b8c = np.zeros((2, cch, 2, 512), FP8NP)
        pc = perm[win:].reshape(cch, 512)
        b8c[0, :, 0] = b8_rows[0][pc]
        b8c[0, :, 1] = b8_rows[1][pc]
        b8c[1, :, 0] = b8_rows[2][pc]
        bw_c = np.stack([bh_all[perm[:win]], bl_all[perm[:win]]])
        ta_p = lab[perm[:win]].astype(np.float32)
        in_maps.append(
            {
                "warm": np.full((2, 64), 0.125, FP8NP),
                "gT8": _dr_tile(g8h[perm]),
                "gT8l": _dr_tile(g8l[perm[:win]]),
                "fT8": _dr_tile_f(f8h),
                "fT8l": _dr_tile_f(f8l),
                "ones8": np.ascontiguousarray(ones8_host.reshape(2, 256).astype(FP8NP)),
                "b8": np.ascontiguousarray(b8c.reshape(2, cch * 1024)),
                "ones2": np.ascontiguousarray(ones2_host.astype(BFNP)),
                "bw": np.ascontiguousarray(bw_c.astype(BFNP)),
                "ta": np.ascontiguousarray(np.broadcast_to(ta_p, (128, win))),
                "tvec": np.ascontiguousarray(
                    t_sorted[RPC * c : RPC * (c + 1)].reshape(MT, 128).T.astype(np.float32)
                ),
                "ident": ident,
            }
        )

    # host-side numerator: P_i = 10*(f_i . G_{t_i} - f_i . f_i) - 10*K_i
    # with G_c = sum of batch features of class c + center_c (exact, f64)
    f64 = features.astype(np.float64)
    G = centers1.astype(np.float64).copy()
    np.add.at(G, targets, f64)
    dots = np.einsum("ij,ij->i", f64, G[targets])
    self_dot = np.einsum("ij,ij->i", f64, f64)
    P_half = dots - self_dot                       # sum over matched != self of r

    # narrow correction bands: ch0 matches stay in [woff_m, woff_m+256) of the
    # own-rows chunk, ch1 matches in its first 256 cols
    WOFF = (0, 64, 192, 256)
    woff = WOFF
    if wch != 2:
        woff = None
    else:
        for c in range(NCORES):
            tc_ = t_sorted[RPC * c : RPC * (c + 1)]
            if len(perms[c]) and (RPC + len_matched[c]) > 512 + 256:
                woff = None
                break
            for m in range(MT):
                cmin, cmax = tc_[128 * m], tc_[128 * m + 127]
                lo = np.searchsorted(tc_, cmin, side="left")
                hi = np.searchsorted(tc_, cmax, side="right")
                if not (WOFF[m] <= lo and hi <= WOFF[m] + 256):
                    woff = None
                    break
            if woff is None:
                break

    ncheap_groups = -(-(NCH - wch) // 4)
    host = {"t_sorted": t_sorted, "w_full": w_full, "wch": wch, "woff": woff,
            "ngrp": ncheap_groups + 1, "P_half": P_half[row_perm]}
    return in_maps, host


def _combine(results, host):
    t_sorted = host["t_sorted"]
    w_full = host["w_full"]
    cnt_batch = np.bincount(t_sorted, minlength=C).astype(np.float64)

    ngrp = host["ngrp"]
    wch = host["wch"]
    na = ngrp + wch + 1
    A = np.empty(B)
    S2 = np.empty(B)
    e1s = np.empty(B)
    for c in range(NCORES):
        o = np.asarray(results[c]["out"], np.float64)
        for m in range(MT):
            rs = slice(RPC * c + 128 * m, RPC * c + 128 * (m + 1))
            a = o[:, na * m : na * (m + 1)]
            A[rs] = a[:, 0:ngrp].sum(axis=1)
            S2[rs] = a[:, ngrp : ngrp + wch].sum(axis=1)
            e1s[rs] = a[:, ngrp + wch]

    ws = w_full[t_sorted]
    K = cnt_batch[t_sorted]
    ds_ = 1.0 / (ws - 1.0) - 1.0 / ws
    S = A - e1s + ds_ * ws * (S2 - e1s)
    P = 10.0 * host["P_half"] - 10.0 * K
    val = P / K - np.log(S)
    return np.float32(-val.mean())


def _run(inputs, trace=False, **kw):
    in_maps, host = _prepare(**inputs)
    nc = _get_nc(host["wch"], host["woff"])
    res = run_bass_kernel_spmd(nc, in_maps, list(range(NCORES)), trace=trace, **kw)
    loss = _combine(res.results, host)
    return loss, res


def kernel(**inputs):
    loss, _ = _run(inputs)
    return loss
